# revision 13
# baseline (speedup 1.0000x reference)
"""Paged multi-head attention decode step on 8 trn2 NeuronCores.

Sharding: tensor-parallel over heads. Core c owns heads [4c, 4c+4):
  - rows  [512c, 512(c+1)) of Wq/Wk/Wv  (shipped pre-transposed, k-major)
  - cols  [512c, 512(c+1)) of Wo        (shipped pre-transposed)
  - head-slice of the (gathered, per-sequence) KV cache
Each core computes q/k/v for its heads for all 8 sequences, injects the new
token's k/v into its KV tiles, runs softmax(q K^T / sqrt(d)) V over the valid
context, then a partial output projection out_c = ctx_c @ Wo_c.  The full
output is the sum over cores (done on host).

Layout notes (trn2 partition-base rule: engine APs must start at partition
0/32/64/96, PE psum writes at 0/32/64):
  - scores/attn live as [128 tokens (partition), pair (free)] tiles,
    pair = 4*b + h.  Cross-pair reductions (max/sum over tokens) go through
    PE transposes to [32 pairs, ...] tiles; per-pair scalars are broadcast
    back across partitions with a partition-step-0 DMA.
  - PV uses V tiles as the stationary operand so ctx emerges as
    [128 d, pair] columns, which feeds the Wo matmul directly.

Sequence lengths (positions) are host-known at trace time, so all loop trip
counts are static and the kernel only reads the valid (128-padded) context.
"""

import math

import numpy as np

import concourse.bass as bass
import concourse.mybir as mybir
import concourse.tile as tile
from concourse import bacc
from concourse.bass_utils import run_bass_kernel_spmd
from concourse.masks import make_identity

BLOCK_SIZE = 16
NUM_HEADS = 32
HEAD_DIM = 128
D_MODEL = NUM_HEADS * HEAD_DIM
B = 8
N_CORES = 8
H_LOC = NUM_HEADS // N_CORES          # 4 heads per core
KSLICE = H_LOC * HEAD_DIM             # 512 contraction slice per core
NPAIR = H_LOC * B                     # 32 (seq, head) pairs per core
SCALE = 1.0 / math.sqrt(HEAD_DIM)
NEG_BIG = -3.0e38

_F32 = mybir.dt.float32


def _cfg_from_positions(pos):
    pos = [int(p) for p in pos]
    tpad = [((p + 1) + 127) // 128 * 128 for p in pos]
    nt = [t // 128 for t in tpad]
    kofs = np.concatenate([[0], np.cumsum([4 * t for t in tpad])]).tolist()
    vofs = np.concatenate([[0], np.cumsum(tpad)]).tolist()
    return {
        "pos": pos, "tpad": tpad, "nt": nt,
        "kofs": kofs, "vofs": vofs,
        "sumk": int(kofs[-1]), "sumv": int(vofs[-1]),
        "maxnt": max(nt),
    }


def _bcast_pairs(nc, psp, const, col, ones, ident, name):
    """[NPAIR,1] column -> [128, NPAIR] sbuf tile with the value of pair j
    replicated down all 128 partitions of column j (via PE transpose + ones
    outer-product)."""
    t1 = psp.tile([1, NPAIR], _F32, tag="ps", name=f"{name}_t1")
    nc.tensor.transpose(t1[:], col[:], ident[0:NPAIR, 0:NPAIR])
    row = const.tile([1, NPAIR], _F32, tag=f"{name}_row", name=f"{name}_row")
    nc.vector.tensor_copy(out=row[:], in_=t1[:])
    t2 = psp.tile([128, NPAIR], _F32, tag="ps", name=f"{name}_t2")
    nc.tensor.matmul(t2[:], lhsT=ones[:], rhs=row[:], start=True, stop=True)
    bc = const.tile([128, NPAIR], _F32, tag=f"{name}_bc", name=f"{name}_bc")
    nc.vector.tensor_copy(out=bc[:], in_=t2[:])
    return bc


def _build(cfg):
    pos, tpad, nt = cfg["pos"], cfg["tpad"], cfg["nt"]
    kofs, vofs = cfg["kofs"], cfg["vofs"]
    maxnt = cfg["maxnt"]

    nc = bacc.Bacc("TRN2", target_bir_lowering=False, debug=False)

    xt_d = nc.dram_tensor("xt", [32, 128, B], _F32, kind="ExternalInput")
    wq_d = nc.dram_tensor("wq_t", [32, 128, KSLICE], _F32, kind="ExternalInput")
    wk_d = nc.dram_tensor("wk_t", [32, 128, KSLICE], _F32, kind="ExternalInput")
    wv_d = nc.dram_tensor("wv_t", [32, 128, KSLICE], _F32, kind="ExternalInput")
    wo_d = nc.dram_tensor("wo_t", [H_LOC, 128, D_MODEL], _F32, kind="ExternalInput")
    kt_d = nc.dram_tensor("kt", [128, cfg["sumk"]], _F32, kind="ExternalInput")
    vg_d = nc.dram_tensor("vg", [cfg["sumv"], KSLICE], _F32, kind="ExternalInput")
    out_d = nc.dram_tensor("out_part", [B, D_MODEL], _F32, kind="ExternalOutput")

    with tile.TileContext(nc) as tc:
        with (
            tc.tile_pool(name="const", bufs=1) as const,
            tc.tile_pool(name="wstream", bufs=3) as wpool,
            tc.tile_pool(name="wostream", bufs=4) as wopool,
            tc.tile_pool(name="kstream", bufs=4) as kpool,
            tc.tile_pool(name="vstream", bufs=4) as vpool,
            tc.tile_pool(name="ps", bufs=8, space="PSUM") as psp,
        ):
            ident = const.tile([128, 128], _F32, tag="ident")
            make_identity(nc, ident[:])
            ones = const.tile([1, 128], _F32, tag="ones")
            nc.vector.memset(ones[:], 1.0)

            xt_sb = const.tile([128, 32, B], _F32, tag="xt")
            nc.sync.dma_start(out=xt_sb[:], in_=xt_d.ap().rearrange("c p b -> p c b"))

            # ---- QKV projections: psum[b, j] = sum_k x[b,k] W[j,k], j local 512
            qkv_sb = {}
            for wname, w_d in (("q", wq_d), ("k", wk_d), ("v", wv_d)):
                ps = psp.tile([B, KSLICE], _F32, tag="ps", name=f"ps_{wname}")
                for g in range(8):
                    wt = wpool.tile([128, 4, KSLICE], _F32, tag="w", name=f"wt_{wname}{g}")
                    nc.sync.dma_start(
                        out=wt[:], in_=w_d.ap()[4 * g : 4 * g + 4].rearrange("c p f -> p c f")
                    )
                    for j in range(4):
                        i = 4 * g + j
                        nc.tensor.matmul(
                            ps[:], lhsT=xt_sb[:, i, :], rhs=wt[:, j, :],
                            start=(i == 0), stop=(i == 31),
                        )
                sb = const.tile([B, KSLICE], _F32, tag=f"{wname}_sb", name=f"{wname}_sb")
                nc.scalar.copy(out=sb[:], in_=ps[:])
                qkv_sb[wname] = sb

            # ---- q,k -> [128 d, pair] column layout (pair = 4b+h), q pre-scaled
            qT = const.tile([128, NPAIR], _F32, tag="qT")
            kT = const.tile([128, NPAIR], _F32, tag="kT")
            for name, dst in (("q", qT), ("k", kT)):
                src = qkv_sb[name]
                for h in range(H_LOC):
                    tp = psp.tile([128, B], _F32, tag="ps", name=f"tp_{name}{h}")
                    nc.tensor.transpose(
                        tp[:], src[0:B, 128 * h : 128 * (h + 1)], ident[0:B, 0:B]
                    )
                    # dst columns {4b+h : b} == view [p (b h)] -> [p h b] at index h
                    nc.vector.tensor_copy(
                        out=dst[:].rearrange("p (b h) -> p h b", h=H_LOC)[:, h, :],
                        in_=tp[:],
                    )
            nc.vector.tensor_scalar_mul(qT[:], qT[:], SCALE)
            v_sb = qkv_sb["v"]

            # ---- scores: per token-tile psum [128 tok, NPAIR], col = 4b+h.
            # One psum region-group per tile: first matmul start=True (zeroes
            # the 2KB region), every column is written exactly once.
            # scores_all[t, tt, pair] in SBUF; slots with tt >= nt[b] = NEG_BIG
            scores_all = const.tile([128, maxnt, NPAIR], _F32, tag="scores")
            nc.gpsimd.memset(scores_all[:], NEG_BIG)
            kt_tiles = {}  # (b, g) -> sbuf tile holding 512 tokens x 4 heads of K^T
            for tt in range(maxnt):
                sc = psp.tile([128, NPAIR], _F32, tag="ps", name=f"sc{tt}")
                bs = [b for b in range(B) if tt < nt[b]]
                for b in bs:
                    g = tt // 4
                    if (b, g) not in kt_tiles:
                        w = min(512, tpad[b] - 512 * g)
                        kt_t = kpool.tile([128, H_LOC, 512], _F32, tag="kt",
                                          name=f"kt{b}_{g}")
                        src = (
                            kt_d.ap()[:, kofs[b] : kofs[b] + 4 * tpad[b]]
                            .rearrange("p (h t) -> p h t", h=H_LOC)
                            [:, :, 512 * g : 512 * g + w]
                        )
                        nc.sync.dma_start(out=kt_t[:, :, 0:w], in_=src)
                        if pos[b] // 512 == g:
                            off = pos[b] - 512 * g
                            nc.vector.tensor_copy(
                                out=kt_t[:, :, off], in_=kT[:, 4 * b : 4 * b + 4]
                            )
                        kt_tiles[(b, g)] = kt_t
                    kt_t = kt_tiles[(b, g)]
                    j0 = 128 * (tt % 4)
                    for h in range(H_LOC):
                        pr = 4 * b + h
                        nc.tensor.matmul(
                            sc[:, pr : pr + 1],
                            lhsT=kt_t[:, h, j0 : j0 + 128],
                            rhs=qT[:, pr : pr + 1],
                            start=(b == bs[0] and h == 0),
                            stop=(b == bs[-1] and h == H_LOC - 1),
                        )
                for b in bs:
                    nc.vector.tensor_copy(
                        out=scores_all[:, tt, 4 * b : 4 * b + 4],
                        in_=sc[:, 4 * b : 4 * b + 4],
                    )

            # ---- softmax max: per-pair global max -> -max broadcast to all parts
            pmax = const.tile([128, NPAIR], _F32, tag="pmax")
            nc.vector.reduce_max(
                out=pmax[:],
                in_=scores_all[:].rearrange("p c j -> p j c"),
                axis=mybir.AxisListType.X,
            )
            pmax_t = psp.tile([NPAIR, 128], _F32, tag="ps", name="pmax_t")
            nc.tensor.transpose(pmax_t[:], pmax[:], ident[:])
            gmax = const.tile([NPAIR, 1], _F32, tag="gmax")
            nc.vector.reduce_max(out=gmax[:], in_=pmax_t[:], axis=mybir.AxisListType.X)
            negmax = const.tile([NPAIR, 1], _F32, tag="negmax")
            nc.vector.tensor_scalar_mul(negmax[:], gmax[:], -1.0)
            nm_bc = _bcast_pairs(nc, psp, const, negmax, ones, ident, "nm")

            # ---- exp (+mask of the partial last token-tile per seq)
            attnT = const.tile([128, maxnt, NPAIR], _F32, tag="attnT")
            for tt in range(maxnt):
                nc.vector.tensor_add(attnT[:, tt, :], scores_all[:, tt, :], nm_bc[:])
                nc.scalar.activation(
                    out=attnT[:, tt, :], in_=attnT[:, tt, :],
                    func=mybir.ActivationFunctionType.Exp,
                )
            for b in range(B):
                r = pos[b] % 128
                if r == 127:
                    continue  # last tile fully valid
                # zero rows p > r of the last tile's 4 columns: keep where r-p >= 0
                nc.gpsimd.affine_select(
                    out=attnT[:, nt[b] - 1, 4 * b : 4 * b + 4],
                    in_=attnT[:, nt[b] - 1, 4 * b : 4 * b + 4],
                    compare_op=mybir.AluOpType.is_ge,
                    fill=0.0,
                    base=r,
                    pattern=[[0, H_LOC]],
                    channel_multiplier=-1,
                )

            # ---- denominators -> 1/sum broadcast, normalize attnT in place
            psums = const.tile([128, NPAIR], _F32, tag="psums")
            nc.vector.reduce_sum(
                out=psums[:],
                in_=attnT[:].rearrange("p c j -> p j c"),
                axis=mybir.AxisListType.X,
            )
            psums_t = psp.tile([NPAIR, 128], _F32, tag="ps", name="psums_t")
            nc.tensor.transpose(psums_t[:], psums[:], ident[:])
            denom = const.tile([NPAIR, 1], _F32, tag="denom")
            nc.vector.reduce_sum(out=denom[:], in_=psums_t[:], axis=mybir.AxisListType.X)
            recip = const.tile([NPAIR, 1], _F32, tag="recip")
            nc.vector.reciprocal(recip[:], denom[:])
            rc_bc = _bcast_pairs(nc, psp, const, recip, ones, ident, "rc")
            for tt in range(maxnt):
                nc.vector.tensor_mul(attnT[:, tt, :], attnT[:, tt, :], rc_bc[:])

            # ---- PV: per seq, ct_b[h, (h',d)] = sum_t attn[t, 4b+h] V[t, (h',d)]
            # (single accumulation group per seq; diagonal h==h' blocks kept)
            ctxT = const.tile([128, NPAIR], _F32, tag="ctxT")  # col = 8h+b
            for b in range(B):
                ct = psp.tile([H_LOC, KSLICE], _F32, tag="ps", name=f"ct{b}")
                for g in range((nt[b] + 3) // 4):
                    r0 = vofs[b] + 512 * g
                    nrow = min(512, tpad[b] - 512 * g)
                    vt = vpool.tile([128, 4, KSLICE], _F32, tag="v", name=f"vt{b}_{g}")
                    nc.sync.dma_start(
                        out=vt[:, 0 : nrow // 128, :],
                        in_=vg_d.ap()[r0 : r0 + nrow].rearrange("(c p) f -> p c f", p=128),
                    )
                    if pos[b] // 512 == g:  # new-token v lands in this tile group
                        nc.sync.dma_start(
                            out=vt[pos[b] % 128 : pos[b] % 128 + 1, (pos[b] // 128) % 4, :],
                            in_=v_sb[b : b + 1, :],
                        )
                    for jj in range(nrow // 128):
                        tt = 4 * g + jj
                        nc.tensor.matmul(
                            ct[:],
                            lhsT=attnT[:, tt, 4 * b : 4 * b + 4],
                            rhs=vt[:, jj, :],
                            start=(tt == 0), stop=(tt == nt[b] - 1),
                        )
                ct_sb = const.tile([H_LOC, KSLICE], _F32, tag="ct_sb", name=f"ct_sb{b}",
                                   bufs=2)
                nc.vector.tensor_copy(out=ct_sb[:], in_=ct[:])
                for h in range(H_LOC):
                    ctt = psp.tile([128, H_LOC], _F32, tag="ps", name=f"ctt{b}_{h}")
                    nc.tensor.transpose(
                        ctt[:], ct_sb[0:H_LOC, 128 * h : 128 * (h + 1)],
                        ident[0:H_LOC, 0:H_LOC],
                    )
                    nc.vector.tensor_copy(
                        out=ctxT[:, 8 * h + b : 8 * h + b + 1], in_=ctt[:, h : h + 1]
                    )

            # ---- output projection partial: out[b, :] = sum_h ctxT[:, 8h+b]^T Wo[h]
            wo_sb = []
            for h in range(H_LOC):
                wt = wopool.tile([128, D_MODEL], _F32, tag="wo", name=f"wo{h}")
                nc.sync.dma_start(out=wt[:], in_=wo_d.ap()[h])
                wo_sb.append(wt)
            outsb = const.tile([B, D_MODEL], _F32, tag="outsb")
            for n in range(8):
                op = psp.tile([B, 512], _F32, tag="ps", name=f"op{n}")
                for h in range(H_LOC):
                    nc.tensor.matmul(
                        op[:],
                        lhsT=ctxT[:, 8 * h : 8 * h + B],
                        rhs=wo_sb[h][:, 512 * n : 512 * (n + 1)],
                        start=(h == 0), stop=(h == H_LOC - 1),
                    )
                nc.scalar.copy(out=outsb[:, 512 * n : 512 * (n + 1)], in_=op[:])
            nc.sync.dma_start(out=out_d.ap(), in_=outsb[:])

    nc.compile()
    return nc


_PROGRAM_CACHE = {}


def _get_program(cfg):
    key = tuple(cfg["pos"])
    if key not in _PROGRAM_CACHE:
        _PROGRAM_CACHE[key] = _build(cfg)
    return _PROGRAM_CACHE[key]


def make_core_inputs(cfg, c, x, Wq, Wk, Wv, Wo, key_cache, value_cache, block_tables):
    """Host-side shard prep for core c (also used by the sim test)."""
    pos, tpad = cfg["pos"], cfg["tpad"]
    h0 = H_LOC * c
    xt = np.ascontiguousarray(
        x.reshape(B, D_MODEL).T.reshape(32, 128, B), dtype=np.float32
    )
    wq_t = np.ascontiguousarray(
        Wq[KSLICE * c : KSLICE * (c + 1), :].T.reshape(32, 128, KSLICE), dtype=np.float32
    )
    wk_t = np.ascontiguousarray(
        Wk[KSLICE * c : KSLICE * (c + 1), :].T.reshape(32, 128, KSLICE), dtype=np.float32
    )
    wv_t = np.ascontiguousarray(
        Wv[KSLICE * c : KSLICE * (c + 1), :].T.reshape(32, 128, KSLICE), dtype=np.float32
    )
    wo_t = np.ascontiguousarray(
        Wo[:, KSLICE * c : KSLICE * (c + 1)].T.reshape(H_LOC, 128, D_MODEL),
        dtype=np.float32,
    )
    kt = np.empty((128, cfg["sumk"]), dtype=np.float32)
    vg = np.empty((cfg["sumv"], KSLICE), dtype=np.float32)
    for b in range(B):
        nb = tpad[b] // BLOCK_SIZE
        blocks = np.asarray(block_tables[b, :nb])
        kb = key_cache[blocks][:, :, h0 : h0 + H_LOC, :].reshape(tpad[b], H_LOC, HEAD_DIM)
        vb = value_cache[blocks][:, :, h0 : h0 + H_LOC, :].reshape(tpad[b], H_LOC, HEAD_DIM)
        kt[:, cfg["kofs"][b] : cfg["kofs"][b] + 4 * tpad[b]] = (
            kb.transpose(2, 1, 0).reshape(HEAD_DIM, H_LOC * tpad[b])
        )
        vg[cfg["vofs"][b] : cfg["vofs"][b] + tpad[b]] = vb.reshape(tpad[b], KSLICE)
    return {
        "xt": xt, "wq_t": wq_t, "wk_t": wk_t, "wv_t": wv_t, "wo_t": wo_t,
        "kt": kt, "vg": vg,
    }


def kernel(x, Wq, Wk, Wv, Wo, key_cache, value_cache, block_tables, positions,
           _trace=False):
    x = np.asarray(x, dtype=np.float32)
    Wq = np.asarray(Wq, dtype=np.float32)
    Wk = np.asarray(Wk, dtype=np.float32)
    Wv = np.asarray(Wv, dtype=np.float32)
    Wo = np.asarray(Wo, dtype=np.float32)
    key_cache = np.asarray(key_cache, dtype=np.float32)
    value_cache = np.asarray(value_cache, dtype=np.float32)
    block_tables = np.asarray(block_tables)
    positions = np.asarray(positions)

    cfg = _cfg_from_positions(positions)
    nc = _get_program(cfg)

    in_maps = [
        make_core_inputs(cfg, c, x, Wq, Wk, Wv, Wo, key_cache, value_cache, block_tables)
        for c in range(N_CORES)
    ]
    res = run_bass_kernel_spmd(nc, in_maps, core_ids=list(range(N_CORES)))
    out = np.zeros((B, D_MODEL), dtype=np.float32)
    for r in res.results:
        out += r["out_part"]
    kernel.last_results = res
    return out.reshape(B, 1, D_MODEL).astype(np.float32)
